# revision 5
# baseline (speedup 1.0000x reference)
"""Trainium2 Bass kernel for nn_BusinessCostLoss (weighted binary CE loss).

Math (per task, per element):
    d    = l1 - l0
    u    = zf * d                 where zf = 2*y - 1 in {-1,+1}
    base = -log(softmax(l)[y]) = log(1 + exp(-u))   (eps=1e-8 dropped: <1e-6 effect on mean)
    pred = 1{l1 > l0}
    w    = 0.1 if pred==y else (1.0 if y==0 else 5.0)
    loss = w * base ;  output means of loss per task + weighted total.

Device trick: w/2 = max(mh, 0.05) with mh = (q2 - 0.5) * rz3,
    q2 = 1{u > 0}, rz3 = -2*zf - 3  (in {-1 (y=0), -5 (y=1)}).
Per-partition row sums come free via scalar_tensor_tensor's accum_out;
host does the final cross-partition/core reduction in float64 and
multiplies by 2 (the /2 fold keeps the weight chain in 2 DVE ops).

Sharding: pure data-parallel. Batch 8388608 = 8 cores x 128 partitions x 8192.
Host sends, per core per task: l0, l1 (deinterleaved bf16 planes) and zf (bf16).
"""

import os

import numpy as np
import ml_dtypes

import concourse.bacc as bacc
import concourse.mybir as mybir
from concourse import tile
from concourse.bass_utils import run_bass_kernel_spmd

B = 8388608
N_CORES = 8
P = 128                      # SBUF partitions
COLS = B // (N_CORES * P)    # 8192 free-dim elements per partition per task
FREE = 2048                  # tile free-dim
NT = COLS // FREE            # tiles per task
TASKS = 3

BF16 = mybir.dt.bfloat16
F32 = mybir.dt.float32
AF = mybir.ActivationFunctionType
OP = mybir.AluOpType

# exposed for test.py (harness ignores)
LAST_RESULTS = None


def _build_nc():
    nc = bacc.Bacc("TRN2")

    ins = {}
    for t in range(TASKS):
        for nm in ("l0", "l1", "zf"):
            ins[(t, nm)] = nc.dram_tensor(f"{nm}_{t}", [P, COLS], BF16, kind="ExternalInput")
    out = nc.dram_tensor("acc_out", [TASKS, P, NT], F32, kind="ExternalOutput")

    with tile.TileContext(nc) as tc:
        with (
            tc.tile_pool(name="io", bufs=3) as io,
            tc.tile_pool(name="mid", bufs=2) as mid,
            tc.tile_pool(name="accp", bufs=4) as accp,
        ):
            for t in range(TASKS):
                for j in range(NT):
                    sl = slice(j * FREE, (j + 1) * FREE)
                    l0 = io.tile([P, FREE], BF16, tag="l0")
                    l1 = io.tile([P, FREE], BF16, tag="l1")
                    zf = io.tile([P, FREE], BF16, tag="zf")
                    nc.sync.dma_start(out=l0[:], in_=ins[(t, "l0")][:, sl])
                    nc.sync.dma_start(out=l1[:], in_=ins[(t, "l1")][:, sl])
                    nc.sync.dma_start(out=zf[:], in_=ins[(t, "zf")][:, sl])

                    d = mid.tile([P, FREE], BF16, tag="d")
                    u = mid.tile([P, FREE], BF16, tag="u")
                    e = mid.tile([P, FREE], BF16, tag="e")
                    base = mid.tile([P, FREE], BF16, tag="base")
                    rz3 = mid.tile([P, FREE], BF16, tag="rz3")
                    q2 = mid.tile([P, FREE], BF16, tag="q2")
                    mh = mid.tile([P, FREE], BF16, tag="mh")
                    lossh = mid.tile([P, FREE], BF16, tag="lossh")
                    acc = accp.tile([P, 1], F32, tag="acc")

                    nc.vector.tensor_sub(out=d[:], in0=l1[:], in1=l0[:])
                    nc.vector.tensor_mul(out=u[:], in0=zf[:], in1=d[:])
                    # base = log(1 + exp(-u))
                    nc.scalar.activation(e[:], u[:], AF.Exp, bias=0.0, scale=-1.0)
                    nc.scalar.activation(base[:], e[:], AF.Ln, bias=1.0, scale=1.0)
                    # rz3 = -2*zf - 3  (label-derived cost coefficient)
                    nc.gpsimd.tensor_scalar(rz3[:], zf[:], -2.0, -3.0, OP.mult, OP.add)
                    # q2 = 1{u > 0}
                    nc.vector.tensor_scalar(q2[:], u[:], 0.0, None, OP.is_gt)
                    # mh = (q2 - 0.5) * rz3  == w-ish/2 signed
                    nc.vector.scalar_tensor_tensor(
                        out=mh[:], in0=q2[:], scalar=0.5, in1=rz3[:],
                        op0=OP.subtract, op1=OP.mult,
                    )
                    # lossh = max(mh, 0.05) * base ; acc = row-sums(lossh)
                    nc.vector.scalar_tensor_tensor(
                        out=lossh[:], in0=mh[:], scalar=0.05, in1=base[:],
                        op0=OP.max, op1=OP.mult, accum_out=acc[:],
                    )
                    nc.sync.dma_start(out=out[t, :, j : j + 1], in_=acc[:])

    # Bacc defers register allocation to finalize(); the axon PJRT path
    # serializes the BIR without finalizing, so do it here.
    if not nc.is_finalized():
        nc.finalize()
    return nc


_NC_CACHE = None


def _get_nc():
    global _NC_CACHE
    if _NC_CACHE is None:
        _NC_CACHE = _build_nc()
    return _NC_CACHE


def _prep_task(logits: np.ndarray, targets: np.ndarray):
    """Host-side layout/dtype prep: deinterleave logit planes and recode
    labels to zf = 2y-1; shard to [N_CORES, P, COLS]."""
    bf = ml_dtypes.bfloat16
    l0 = logits[:, 0].astype(bf).reshape(N_CORES, P, COLS)
    l1 = logits[:, 1].astype(bf).reshape(N_CORES, P, COLS)
    zf = (2 * targets.astype(np.int8) - 1).astype(bf).reshape(N_CORES, P, COLS)
    return l0, l1, zf


def kernel(logits_a, logits_b, logits_c, targets_a, targets_b, targets_c) -> np.ndarray:
    global LAST_RESULTS
    nc = _get_nc()

    planes = [
        _prep_task(np.asarray(logits_a), np.asarray(targets_a)),
        _prep_task(np.asarray(logits_b), np.asarray(targets_b)),
        _prep_task(np.asarray(logits_c), np.asarray(targets_c)),
    ]

    in_maps = []
    for c in range(N_CORES):
        m = {}
        for t in range(TASKS):
            l0, l1, zf = planes[t]
            m[f"l0_{t}"] = l0[c]
            m[f"l1_{t}"] = l1[c]
            m[f"zf_{t}"] = zf[c]
        in_maps.append(m)

    want_trace = bool(os.environ.get("BASS_TRACE"))
    if want_trace:
        try:  # tracing needs the axon NTFF hook module; degrade if absent
            import antenv.axon_hooks  # noqa: F401
        except ImportError:
            want_trace = False
            os.environ["BASS_NEVER_TRACE"] = "1"

    res = run_bass_kernel_spmd(
        nc,
        in_maps,
        list(range(N_CORES)),
        trace=want_trace,
    )
    LAST_RESULTS = res

    sums = np.zeros(TASKS, dtype=np.float64)
    for c in range(N_CORES):
        acc = np.asarray(res.results[c]["acc_out"], dtype=np.float64)  # [TASKS, P, NT]
        sums += acc.sum(axis=(1, 2))
    means = 2.0 * sums / B  # x2 undoes the /2 weight fold
    la, lb, lc = means
    total = 1.0 * la + 0.5 * lb + 2.0 * lc
    return np.array([la, lb, lc, total], dtype=np.float32)
